# revision 7
# baseline (speedup 1.0000x reference)
"""BitNet-style binary linear: y = x @ w_q.T + bias, w_q = clip(round(w/g))*g.

Strategy (8 NeuronCores, tensor-parallel on out_features):
  - Host: compute g = max(mean|w|, 1e-5); s = clip(rint(w/g), -1, 1). s is
    ternary so it is EXACT in fp16. Fold g into x (xs = g*x) so the device
    matmul needs no rescale; the only precision loss is one fp16 rounding of
    x (~2e-4 relative), with fp32 PSUM accumulation.
  - Shard s rows (out_features) 8-ways; replicate x. Each core computes
    out[8192, 2048] = xs @ s_shard.T + bias_shard with all of s_shard.T
    resident in SBUF (16 MB fp16) and x streamed in r-blocks.
  - matmul(psum[r128, f512], lhsT=xtT[k128, r128], rhs=wT[k128, f512])
    accumulated over 32 k-chunks; psum initialized with bias via a K=1
    ones-matmul; DVE evicts psum -> SBUF; DMA to HBM.
"""

import numpy as np

B, S, D_IN, D_OUT = 4, 2048, 4096, 16384
N_CORES = 8
R = B * S                 # 8192 rows of x
F = D_OUT // N_CORES      # 2048 features per core
KC = D_IN // 128          # 32 k-chunks
RB = 512                  # r-block (rows per x DMA)
FT = 512                  # f-tile (psum free dim)

_CACHE = {}


def _build_nc():
    import concourse.mybir as mybir
    import concourse.tile as tile
    from concourse import bacc

    fp16 = mybir.dt.float16
    fp8 = mybir.dt.float8e4
    f32 = mybir.dt.float32

    nc = bacc.Bacc("TRN2", target_bir_lowering=False, debug=False,
                   num_devices=N_CORES)
    xt = nc.declare_dram_parameter("xt", [D_IN, R], fp16, isOutput=False)
    wt = nc.declare_dram_parameter("wt", [D_IN, F], fp8, isOutput=False)
    bias = nc.declare_dram_parameter("bias", [1, F], fp16, isOutput=False)
    out = nc.declare_dram_parameter("out", [R, F], f32, isOutput=True)

    xt_ap = xt[:, :].rearrange("(c p) n -> p c n", p=128)  # [128, KC, R]
    wt_ap = wt[:, :].rearrange("(c p) n -> p c n", p=128)  # [128, KC, F]

    NF = F // FT  # f-tiles
    with tile.TileContext(nc) as tc:
        with (
            tc.tile_pool(name="wpool", bufs=1) as wpool,
            tc.tile_pool(name="cpool", bufs=1) as cpool,
            tc.tile_pool(name="xpool", bufs=2) as xpool,
            tc.tile_pool(name="opool", bufs=4) as opool,
            tc.tile_pool(name="pspool", bufs=4, space="PSUM") as pspool,
        ):
            # broadcast bias across partitions once: ones[1,128].T @ bias[1,512]
            bias_sb = cpool.tile([1, F], fp16, tag="bias")
            nc.sync.dma_start(bias_sb[:], bias[:, :])
            ones_sb = cpool.tile([1, 128], fp16, tag="ones")
            nc.gpsimd.memset(ones_sb[:], 1.0)
            bias_bc = cpool.tile([128, F], f32, tag="bias_bc")
            for f in range(NF):
                bp = pspool.tile([128, FT], f32)
                nc.tensor.matmul(bp[:], ones_sb[:],
                                 bias_sb[:, f * FT:(f + 1) * FT],
                                 start=True, stop=True)
                nc.vector.tensor_copy(bias_bc[:, f * FT:(f + 1) * FT], bp[:])

            # resident weights (fp8, ternary-exact), split in 2 f-halves so
            # compute on the first half overlaps the second half's DMA
            FH = F // 2
            wt_sb = {}
            for fh in range(2):
                for c in range(KC):
                    t = wpool.tile([128, FH], fp8, tag=f"w{c}_{fh}")
                    nc.sync.dma_start(t[:], wt_ap[:, c, fh * FH:(fh + 1) * FH])
                    wt_sb[(c, fh)] = t

            # r-blocks: small first blocks prime the pipeline while wt loads
            blocks = [(0, 128), (128, 384)]
            r0_ = 512
            while r0_ < R:
                blocks.append((r0_, RB))
                r0_ += RB

            for rb0, rbn in blocks:
                xt_t = xpool.tile([128, KC, RB], fp16)
                nc.sync.dma_start(xt_t[:, :, :rbn], xt_ap[:, :, rb0:rb0 + rbn])
                for rt in range(rbn // 128):
                    r0 = rb0 + rt * 128
                    for f in range(NF):
                        fh, fo = divmod(f * FT, FH)
                        ps = pspool.tile([128, FT], f32)
                        for c in range(KC):
                            nc.tensor.matmul(
                                ps[:],
                                xt_t[:, c, rt * 128:(rt + 1) * 128],
                                wt_sb[(c, fh)][:, fo:fo + FT],
                                start=(c == 0), stop=(c == KC - 1),
                            )
                        ob = opool.tile([128, FT], f32)
                        nc.vector.tensor_add(
                            ob[:], ps[:], bias_bc[:, f * FT:(f + 1) * FT]
                        )
                        nc.sync.dma_start(
                            out[r0:r0 + 128, f * FT:(f + 1) * FT], ob[:]
                        )
    nc.compile()
    return nc


def _prepare_in_maps(x, weight, bias):
    x = np.asarray(x)
    weight = np.asarray(weight)
    bias = np.asarray(bias)

    gamma = np.float32(max(np.mean(np.abs(weight), dtype=np.float64), 1e-5))
    s = np.clip(np.rint(weight.astype(np.float32) / gamma), -1.0, 1.0)

    import ml_dtypes

    xs = (x.reshape(R, D_IN) * gamma).astype(np.float32)
    xt = np.ascontiguousarray(xs.T).astype(np.float16)          # [D_IN, R]
    st = np.ascontiguousarray(s.T).astype(ml_dtypes.float8_e4m3)  # [D_IN, D_OUT]
    b16 = bias.astype(np.float16)

    in_maps = []
    for c in range(N_CORES):
        in_maps.append({
            "xt": xt,
            "wt": np.ascontiguousarray(st[:, c * F:(c + 1) * F]),
            "bias": np.ascontiguousarray(b16[c * F:(c + 1) * F]).reshape(1, F),
        })
    return in_maps


def _assemble(results):
    out = np.concatenate([results[c]["out"] for c in range(N_CORES)], axis=1)
    return out.reshape(B, S, D_OUT)


def kernel(x, weight, bias):
    from concourse.bass_utils import run_bass_kernel_spmd

    in_maps = _prepare_in_maps(x, weight, bias)
    if "nc" not in _CACHE:
        _CACHE["nc"] = _build_nc()
    res = run_bass_kernel_spmd(_CACHE["nc"], in_maps, core_ids=list(range(N_CORES)))
    return _assemble(res.results)
